# revision 17
# baseline (speedup 1.0000x reference)
"""GAT (2-head) + 3x dense/LayerNorm + pairwise-distance kernel for 8 TRN2 NeuronCores.

Strategy: dst-sharded edge processing (one-hot matmul aggregation with the
softmax weights applied on the matmul RHS), bf16 gather table and matmuls,
replicated small dense weights, row-block-sharded NxN cdist in bf16 with an
exact split-fp32 sq trick, bf16 output converted to f32 on host.
"""
import sys

import numpy as np

# Environment bootstrap (harness may run from a bare directory).
for _p in ("/root/.axon_site", "/root/.axon_site/_ro/trn_rl_repo",
           "/root/.axon_site/_ro/pypackages", "/opt/trn_rl_repo"):
    if _p not in sys.path:
        sys.path.append(_p)

import ml_dtypes
import concourse.bass as bass
import concourse.bacc as bacc
import concourse.mybir as mybir
import concourse.tile as tile
from concourse.masks import make_identity
from concourse.bass_utils import run_bass_kernel_spmd

dt = mybir.dt
OP = mybir.AluOpType
AF = mybir.ActivationFunctionType
BF16 = ml_dtypes.bfloat16

N = 10000
NPAD = 10240          # 80 * 128
NB = 80               # node blocks (phase A)
FIN = 256
F = 128               # per-head GAT dim
H = 2
HROW = 384            # htable row bf16 elems (768B, multiple of 256B)
CORES = 8
SHARD = 1280          # dst rows per core (core 7: 1040 useful)
RB = 10               # dst blocks per core
RPAD = 1280
CCH = 512             # cdist column chunk
MRG = 4               # cdist chunks merged per output DMA
NCOL = 10240          # padded output columns
GCMAX = 6             # max tiles per gather call (768 idxs = 48 desc/engine)
EPS = 1e-5

_BUILD_CACHE = {}
_LAST_RESULTS = None


def _chunks(TB):
    out = []
    t = TB
    while t > 0:
        g = min(GCMAX, t)
        out.append(g)
        t -= g
    return out


def _build(TB, phases="ABCDE"):
    CHUNKS = _chunks(TB)

    nc = bacc.Bacc("TRN2", target_bir_lowering=False, debug=False, num_devices=CORES)

    def din(name, shape, d=dt.float32):
        return nc.dram_tensor(name, shape, d, kind="ExternalInput").ap()

    xt = din("xt", [FIN, NPAD], dt.bfloat16)
    wg = din("wg", [FIN, FIN])
    att_src = din("att_src", [H, F])
    att_dst = din("att_dst", [H, F])
    bgat = din("bgat", [1, FIN])
    wa = din("wa", [128, 256])
    ba = din("ba", [1, 128])
    lnaw = din("lnaw", [1, 128])
    lnab = din("lnab", [1, 128])
    w1 = din("w1", [64, 128])
    b1 = din("b1", [1, 64])
    ln1w = din("ln1w", [1, 64])
    ln1b = din("ln1b", [1, 64])
    w2 = din("w2", [32, 64])
    b2 = din("b2", [1, 32])
    ln2w = din("ln2w", [1, 32])
    ln2b = din("ln2b", [1, 32])
    w3 = din("w3", [3, 32])
    b3 = din("b3", [1, 3])
    hidx = din("hidx", [RB, 128, TB * 8], dt.int16)
    adrw = din("adrw", [RB, 128, 8], dt.int16)
    dstloc = din("dstloc", [RB, 128, TB])
    dstrow = din("dstrow", [RB, 1, TB * 128], dt.bfloat16)
    outD = nc.dram_tensor("outD", [RPAD, NCOL], dt.bfloat16, kind="ExternalOutput").ap()

    class _PhaseDone(Exception):
        pass

    import contextlib
    try:
        _tc_cm = tile.TileContext(nc)
        tc = _tc_cm.__enter__()
        est = contextlib.ExitStack()
        with est:
            top = est.enter_context(tc.tile_pool(name="top", bufs=1))
            dram = est.enter_context(tc.tile_pool(name="dram", bufs=1, space="DRAM"))

            htable = dram.tile([NPAD, HROW], dt.bfloat16, tag="htable")
            cc_in = dram.tile([5, RPAD], dt.bfloat16, tag="cc_in")
            cc_out = dram.tile([CORES, 5, RPAD], dt.bfloat16, tag="cc_out")

            ident = top.tile([128, 128], dt.float32, tag="ident")
            make_identity(nc, ident[:])
            iota_i = top.tile([128, 128], dt.int32, tag="iota_i")
            nc.gpsimd.iota(iota_i[:], pattern=[[1, 128]], base=0, channel_multiplier=0)
            iota_f = top.tile([128, 128], dt.float32, tag="iota_f")
            nc.vector.tensor_copy(out=iota_f[:], in_=iota_i[:])
            ones_row = top.tile([1, 128], dt.float32, tag="ones_row")
            nc.vector.memset(ones_row[:], 1.0)
            ones_row_b = top.tile([1, 128], dt.bfloat16, tag="ones_row_b")
            nc.vector.memset(ones_row_b[:], 1.0)
            iotac_i = top.tile([128, 1], dt.int32, tag="iotac_i")
            nc.gpsimd.iota(iotac_i[:], pattern=[[1, 1]], base=0, channel_multiplier=1)
            iota_c = top.tile([128, 1], dt.float32, tag="iota_c")
            nc.vector.tensor_copy(out=iota_c[:], in_=iotac_i[:])
            eps_col = top.tile([128, 1], dt.float32, tag="eps_col")
            nc.vector.memset(eps_col[:], EPS)

            # ---- weight prep ----
            with tc.tile_pool(name="wprep", bufs=1) as wp, \
                 tc.tile_pool(name="wpsum", bufs=2, space="PSUM") as wps:
                wg0 = wp.tile([128, 256], dt.float32, tag="wg0")
                wg1 = wp.tile([128, 256], dt.float32, tag="wg1")
                nc.sync.dma_start(out=wg0[:], in_=wg[0:128, :])
                nc.sync.dma_start(out=wg1[:], in_=wg[128:256, :])
                # rhs chunks [128, 260] bf16: cols 0:256 = WgT[k,:], 256:260 = [as0,as1,ad0,ad1]
                rhsA0 = top.tile([128, 260], dt.bfloat16, tag="rhsA0")
                rhsA1 = top.tile([128, 260], dt.bfloat16, tag="rhsA1")
                for (ko, rhs_t) in ((0, rhsA0), (1, rhsA1)):
                    for oo, wgt in ((0, wg0), (1, wg1)):
                        p = wps.tile([128, 128], dt.float32, tag="tp", space="PSUM")
                        nc.tensor.transpose(out=p[:], in_=wgt[:, ko * 128:(ko + 1) * 128],
                                            identity=ident[:])
                        nc.scalar.copy(out=rhs_t[:, oo * 128:(oo + 1) * 128], in_=p[:])
                # attention fold: wtil[c, k] = sum_j attdiag[j, c] * Wg[j, k]
                asb = wp.tile([2, 128], dt.float32, tag="asb")
                adb = wp.tile([2, 128], dt.float32, tag="adb")
                nc.sync.dma_start(out=asb[:], in_=att_src[:])
                nc.sync.dma_start(out=adb[:], in_=att_dst[:])
                asT = wps.tile([128, 2], dt.float32, tag="attp", space="PSUM")
                nc.tensor.transpose(out=asT[:], in_=asb[:], identity=ident[:2, :2])
                adT = wps.tile([128, 2], dt.float32, tag="attp", space="PSUM")
                nc.tensor.transpose(out=adT[:], in_=adb[:], identity=ident[:2, :2])
                attd0 = wp.tile([128, 4], dt.float32, tag="attd0")
                attd1 = wp.tile([128, 4], dt.float32, tag="attd1")
                nc.vector.memset(attd0[:], 0.0)
                nc.vector.memset(attd1[:], 0.0)
                nc.scalar.copy(out=attd0[:, 0:1], in_=asT[:, 0:1])
                nc.scalar.copy(out=attd0[:, 2:3], in_=adT[:, 0:1])
                nc.scalar.copy(out=attd1[:, 1:2], in_=asT[:, 1:2])
                nc.scalar.copy(out=attd1[:, 3:4], in_=adT[:, 1:2])
                wtilp = wps.tile([4, 256], dt.float32, tag="wtilp", space="PSUM")
                nc.tensor.matmul(wtilp[:], attd0[:], wg0[:], start=True, stop=False)
                nc.tensor.matmul(wtilp[:], attd1[:], wg1[:], start=False, stop=True)
                wtil = wp.tile([4, 256], dt.float32, tag="wtil")
                nc.scalar.copy(out=wtil[:], in_=wtilp[:])
                for ko, rhs_t in ((0, rhsA0), (1, rhsA1)):
                    p = wps.tile([128, 4], dt.float32, tag="attp", space="PSUM")
                    nc.tensor.transpose(out=p[:], in_=wtil[:, ko * 128:(ko + 1) * 128],
                                        identity=ident[:4, :4])
                    nc.scalar.copy(out=rhs_t[:, 256:260], in_=p[:])

                # dense weight transposes (bf16)
                wa_sb = wp.tile([128, 256], dt.float32, tag="wa_sb")
                nc.sync.dma_start(out=wa_sb[:], in_=wa[:])
                waT0 = top.tile([128, 128], dt.bfloat16, tag="waT0")
                waT1 = top.tile([128, 128], dt.bfloat16, tag="waT1")
                for oo, dst_t in ((0, waT0), (1, waT1)):
                    p = wps.tile([128, 128], dt.float32, tag="tp", space="PSUM")
                    nc.tensor.transpose(out=p[:], in_=wa_sb[:, oo * 128:(oo + 1) * 128],
                                        identity=ident[:])
                    nc.scalar.copy(out=dst_t[:], in_=p[:])
                w1_sb = wp.tile([64, 128], dt.float32, tag="w1_sb")
                nc.sync.dma_start(out=w1_sb[:], in_=w1[:])
                w1T = top.tile([128, 64], dt.bfloat16, tag="w1T")
                p = wps.tile([128, 128], dt.float32, tag="tp", space="PSUM")
                nc.tensor.transpose(out=p[:, 0:64], in_=w1_sb[:], identity=ident[:64, :64])
                nc.scalar.copy(out=w1T[:], in_=p[:, 0:64])
                w2_sb = wp.tile([32, 64], dt.float32, tag="w2_sb")
                nc.sync.dma_start(out=w2_sb[:], in_=w2[:])
                w2T = top.tile([64, 32], dt.bfloat16, tag="w2T")
                p = wps.tile([128, 128], dt.float32, tag="tp", space="PSUM")
                nc.tensor.transpose(out=p[:64, 0:32], in_=w2_sb[:], identity=ident[:32, :32])
                nc.scalar.copy(out=w2T[:], in_=p[:64, 0:32])
                w3_sb = wp.tile([3, 32], dt.float32, tag="w3_sb")
                nc.sync.dma_start(out=w3_sb[:], in_=w3[:])
                w3T = top.tile([32, 3], dt.bfloat16, tag="w3T")
                p = wps.tile([128, 128], dt.float32, tag="tp", space="PSUM")
                nc.tensor.transpose(out=p[:32, 0:3], in_=w3_sb[:], identity=ident[:3, :3])
                nc.scalar.copy(out=w3T[:], in_=p[:32, 0:3])

                # broadcast bias / LN tiles (f32)
                def bcast(vec_ap, n, tag):
                    t = top.tile([128, n], dt.float32, tag=tag)
                    v = wp.tile([1, n], dt.float32, tag="bvec_" + tag)
                    nc.sync.dma_start(out=v[:], in_=vec_ap)
                    p2 = wps.tile([128, 256], dt.float32, tag="bp", space="PSUM")
                    nc.tensor.matmul(p2[:, 0:n], ones_row[:], v[:], start=True, stop=True)
                    nc.scalar.copy(out=t[:], in_=p2[:, 0:n])
                    return t

                bgat_bc = bcast(bgat[:], 256, "bgat_bc")
                ba_bc = bcast(ba[:], 128, "ba_bc")
                lnaw_bc = bcast(lnaw[:], 128, "lnaw_bc")
                lnab_bc = bcast(lnab[:], 128, "lnab_bc")
                b1_bc = bcast(b1[:], 64, "b1_bc")
                ln1w_bc = bcast(ln1w[:], 64, "ln1w_bc")
                ln1b_bc = bcast(ln1b[:], 64, "ln1b_bc")
                b2_bc = bcast(b2[:], 32, "b2_bc")
                ln2w_bc = bcast(ln2w[:], 32, "ln2w_bc")
                ln2b_bc = bcast(ln2b[:], 32, "ln2b_bc")
                b3_bc = bcast(b3[:], 3, "b3_bc")

            # ---- phase A: htable = [h0 | 1 | h1 | 1 | a_s | a_d | junk] per node ----
            with tc.tile_pool(name="pa", bufs=3) as pa, \
                 tc.tile_pool(name="pa_ht", bufs=3) as pa_ht, \
                 tc.tile_pool(name="pa_h", bufs=2, space="PSUM") as pa_h:
                for i in range(NB):
                    r0 = i * 128
                    xt0 = pa.tile([128, 128], dt.bfloat16, tag="xt0")
                    xt1 = pa.tile([128, 128], dt.bfloat16, tag="xt1")
                    nc.sync.dma_start(out=xt0[:], in_=xt[0:128, r0:r0 + 128])
                    nc.sync.dma_start(out=xt1[:], in_=xt[128:256, r0:r0 + 128])
                    hp = pa_h.tile([128, 260], dt.float32, tag="hp", space="PSUM")
                    nc.tensor.matmul(hp[:], xt0[:], rhsA0[:], start=True, stop=False)
                    nc.tensor.matmul(hp[:], xt1[:], rhsA1[:], start=False, stop=True)
                    ht = pa_ht.tile([128, HROW], dt.bfloat16, tag="ht")
                    nc.scalar.copy(out=ht[:, 0:128], in_=hp[:, 0:128])
                    nc.vector.memset(ht[:, 128:129], 1.0)
                    nc.vector.tensor_copy(out=ht[:, 129:257], in_=hp[:, 128:256])
                    nc.vector.memset(ht[:, 257:258], 1.0)
                    nc.scalar.copy(out=ht[:, 258:262], in_=hp[:, 256:260])
                    nc.vector.memset(ht[:, 262:HROW], 0.0)
                    nc.scalar.dma_start(out=htable[r0:r0 + 128, :], in_=ht[:])

            if phases == "A":
                dbg = top.tile([128, HROW], dt.bfloat16, tag="dbgA")
                for i in range(RB):
                    nc.sync.dma_start(out=dbg[:], in_=htable[i * 128:(i + 1) * 128, :])
                    nc.sync.dma_start(out=outD[i * 128:(i + 1) * 128, 0:HROW], in_=dbg[:])
                raise _PhaseDone()

            # ---- phase B: GAT aggregation per dst block ----
            SKIP_AD = "1" in phases
            xg_pool = est.enter_context(tc.tile_pool(name="xg", bufs=1))
            adp_pool = est.enter_context(tc.tile_pool(name="pb_adb", bufs=1))
            xgs = []
            # a_d for own 1280 dst rows: one gather of the htable attention tail

            with tc.tile_pool(name="pb_idx", bufs=2) as pb_idx, \
                 tc.tile_pool(name="pb_g", bufs=3) as pb_g, \
                 tc.tile_pool(name="pb_ex", bufs=3) as pb_ex, \
                 tc.tile_pool(name="pb_oh", bufs=2) as pb_oh, \
                 tc.tile_pool(name="pb_oh2", bufs=4) as pb_oh2, \
                 tc.tile_pool(name="pb_gs", bufs=6) as pb_gs, \
                 tc.tile_pool(name="pb_ep", bufs=2) as pb_ep, \
                 tc.tile_pool(name="pb_ps", bufs=2, space="PSUM") as pb_ps, \
                 tc.tile_pool(name="pb_bc", bufs=2, space="PSUM") as pb_bc, \
                 tc.tile_pool(name="pb_adp", bufs=4, space="PSUM") as pb_adp:
                for b in range(RB):
                    hix = pb_idx.tile([128, TB * 8], dt.int16, tag="hix")
                    nc.sync.dma_start(out=hix[:], in_=hidx[b])
                    dloc = pb_idx.tile([128, TB], dt.float32, tag="dloc")
                    nc.sync.dma_start(out=dloc[:], in_=dstloc[b])
                    drow = pb_idx.tile([1, TB * 128], dt.bfloat16, tag="drow")
                    nc.sync.dma_start(out=drow[:], in_=dstrow[b])
                    adblk = pb_idx.tile([128, 1, HROW], dt.bfloat16, tag="adblk")
                    if "2" in phases:
                        nc.vector.memset(adblk[:], 0.0)
                    elif not SKIP_AD:
                        arw = pb_idx.tile([128, 8], dt.int16, tag="arw")
                        nc.sync.dma_start(out=arw[:], in_=adrw[b])
                        nc.gpsimd.dma_gather(
                            out_ap=adblk[:], in_ap=htable[:],
                            idxs_ap=arw[:], num_idxs=128, num_idxs_reg=128,
                            elem_size=HROW)

                    ps = pb_ps.tile([128, 258], dt.float32, tag="ps", space="PSUM")

                    t0 = 0
                    for gc in CHUNKS:
                        g = pb_g.tile([128, GCMAX, HROW], dt.bfloat16, tag="g")
                        nc.gpsimd.dma_gather(
                            out_ap=g[:, 0:gc, :], in_ap=htable[:],
                            idxs_ap=hix[:, t0 * 8:(t0 + gc) * 8],
                            num_idxs=128 * gc, num_idxs_reg=128 * gc,
                            elem_size=HROW)
                        # ohT[dstlane, edgecol] one-hots for the a_d fetch
                        if not SKIP_AD:
                            ohT = pb_oh.tile([128, GCMAX * 128], dt.bfloat16, tag="ohT")
                            for s0 in range(0, gc * 128, 512):
                                w = min(512, gc * 128 - s0)
                                bcp = pb_bc.tile([128, 512], dt.float32, tag="bcp",
                                                 space="PSUM")
                                nc.tensor.matmul(bcp[:, 0:w], ones_row_b[:],
                                                 drow[0:1, t0 * 128 + s0:t0 * 128 + s0 + w],
                                                 start=True, stop=True)
                                nc.vector.tensor_scalar(
                                    out=ohT[:, s0:s0 + w], in0=bcp[:, 0:w],
                                    scalar1=iota_c[:, 0:1], scalar2=None, op0=OP.is_equal)
                            # a_d per edge via permutation matmuls
                            adp = pb_ex.tile([128, GCMAX, 2], dt.float32, tag="adpE")
                            for tl in range(gc):
                                adpp = pb_adp.tile([128, 2], dt.float32, tag="adp",
                                                   space="PSUM")
                                nc.tensor.matmul(adpp[:],
                                                 ohT[:, tl * 128:(tl + 1) * 128],
                                                 adblk[:, 0, 260:262], start=True, stop=True)
                                nc.scalar.copy(out=adp[:, tl, :], in_=adpp[:])
                        # EX = exp(lrelu02(a_s[src] + a_d[dst])) = max(exp(e), exp(0.2e))
                        asf = pb_ex.tile([128, GCMAX, 2], dt.float32, tag="asf")
                        nc.vector.tensor_copy(out=asf[:, 0:gc, :], in_=g[:, 0:gc, 258:260])
                        exr = pb_ex.tile([128, GCMAX, 2], dt.float32, tag="exr")
                        if SKIP_AD:
                            nc.vector.tensor_copy(out=exr[:, 0:gc, :], in_=asf[:, 0:gc, :])
                        else:
                            nc.vector.tensor_tensor(out=exr[:, 0:gc, :], in0=asf[:, 0:gc, :],
                                                    in1=adp[:, 0:gc, :], op=OP.add)
                        exn = pb_ex.tile([128, GCMAX, 2], dt.float32, tag="exn")
                        nc.scalar.activation(out=exn[:, 0:gc, :], in_=exr[:, 0:gc, :],
                                             func=AF.Exp, scale=0.2)
                        exp_ = pb_ex.tile([128, GCMAX, 2], dt.float32, tag="exp_")
                        nc.scalar.activation(out=exp_[:, 0:gc, :], in_=exr[:, 0:gc, :],
                                             func=AF.Exp)
                        exv = pb_ex.tile([128, GCMAX, 2], dt.float32, tag="exv")
                        nc.vector.tensor_tensor(out=exv[:, 0:gc, :], in0=exp_[:, 0:gc, :],
                                                in1=exn[:, 0:gc, :], op=OP.max)

                        for tl in range(gc):
                            t = t0 + tl
                            gs = pb_gs.tile([128, 258], dt.bfloat16, tag="gs")
                            nc.scalar.activation(out=gs[:, 0:129], in_=g[:, tl, 0:129],
                                                 func=AF.Copy, scale=exv[:, tl, 0:1])
                            nc.scalar.activation(out=gs[:, 129:258], in_=g[:, tl, 129:258],
                                                 func=AF.Copy, scale=exv[:, tl, 1:2])
                            oh = pb_oh2.tile([128, 128], dt.bfloat16, tag="oh")
                            nc.vector.tensor_scalar(
                                out=oh[:], in0=iota_f[:], scalar1=dloc[:, t:t + 1],
                                scalar2=None, op0=OP.is_equal)
                            nc.tensor.matmul(ps[:], oh[:], gs[:],
                                             start=(t == 0), stop=(t == TB - 1))
                        t0 += gc

                    # epilogue: normalize, +b_gat, lrelu(0.01)
                    rec0 = pb_ep.tile([128, 1], dt.float32, tag="rec0")
                    nc.vector.reciprocal(rec0[:], ps[:, 128:129])
                    rec1 = pb_ep.tile([128, 1], dt.float32, tag="rec1")
                    nc.vector.reciprocal(rec1[:], ps[:, 257:258])
                    xg = xg_pool.tile([128, 256], dt.float32, tag=f"xg{b}")
                    nc.scalar.activation(out=xg[:, 0:128], in_=ps[:, 0:128],
                                         func=AF.Copy, scale=rec0[:])
                    nc.scalar.activation(out=xg[:, 128:256], in_=ps[:, 129:257],
                                         func=AF.Copy, scale=rec1[:])
                    nc.vector.tensor_tensor(out=xg[:], in0=xg[:], in1=bgat_bc[:], op=OP.add)
                    ng = pb_ep.tile([128, 256], dt.float32, tag="ng")
                    nc.vector.tensor_scalar(out=ng[:], in0=xg[:], scalar1=0.0,
                                            scalar2=0.01, op0=OP.min, op1=OP.mult)
                    nc.vector.scalar_tensor_tensor(out=xg[:], in0=xg[:], scalar=0.0,
                                                   in1=ng[:], op0=OP.max, op1=OP.add)
                    xgs.append(xg)

            if phases.startswith("AB") and "C" not in phases:
                for b in range(RB):
                    xgb_d = top.tile([128, 256], dt.bfloat16, tag=f"dbgB{b}")
                    nc.vector.tensor_copy(out=xgb_d[:], in_=xgs[b][:])
                    nc.sync.dma_start(out=outD[b * 128:(b + 1) * 128, 0:256], in_=xgb_d[:])
                raise _PhaseDone()

            # ---- phase C: dense + LN on own shard; build cc rows and cdist lhsT ----
            cc_sb = top.tile([5, RPAD], dt.bfloat16, tag="cc_sb")
            lhsT_pool = est.enter_context(tc.tile_pool(name="lhsT", bufs=1))
            lhsTs = []
            with tc.tile_pool(name="pc", bufs=3) as pc, \
                 tc.tile_pool(name="pc_ps", bufs=2, space="PSUM") as pc_ps:

                def layer_norm_lrelu(xin, fdim, bias_bc, w_bc, b_bc):
                    # y = xin + bias; u = LN(y)*w + b; return lrelu001(u)
                    y = pc.tile([128, fdim], dt.float32, tag=f"y{fdim}")
                    nc.vector.tensor_tensor(out=y[:], in0=xin, in1=bias_bc[:], op=OP.add)
                    scr = pc.tile([128, fdim], dt.float32, tag=f"scr{fdim}")
                    msum = pc.tile([128, 1], dt.float32, tag="msum")
                    nc.scalar.activation(out=scr[:], in_=y[:], func=AF.Copy,
                                         accum_out=msum[:])
                    sqs = pc.tile([128, 1], dt.float32, tag="sqs")
                    nc.scalar.activation(out=scr[:], in_=y[:], func=AF.Square,
                                         accum_out=sqs[:])
                    mean = pc.tile([128, 1], dt.float32, tag="mean")
                    nc.vector.tensor_scalar(out=mean[:], in0=msum[:], scalar1=1.0 / fdim,
                                            scalar2=None, op0=OP.mult)
                    var = pc.tile([128, 1], dt.float32, tag="var")
                    nc.vector.tensor_scalar(out=var[:], in0=sqs[:], scalar1=1.0 / fdim,
                                            scalar2=None, op0=OP.mult)
                    m2 = pc.tile([128, 1], dt.float32, tag="m2")
                    nc.vector.tensor_scalar(out=m2[:], in0=mean[:], scalar1=mean[:, 0:1],
                                            scalar2=None, op0=OP.mult)
                    nc.vector.tensor_tensor(out=var[:], in0=var[:], in1=m2[:], op=OP.subtract)
                    sd = pc.tile([128, 1], dt.float32, tag="sd")
                    nc.scalar.activation(out=sd[:], in_=var[:], func=AF.Sqrt, bias=eps_col[:, 0:1])
                    rstd = pc.tile([128, 1], dt.float32, tag="rstd")
                    nc.vector.reciprocal(rstd[:], sd[:])
                    u = pc.tile([128, fdim], dt.float32, tag=f"u{fdim}")
                    nc.vector.scalar_tensor_tensor(out=u[:], in0=y[:], scalar=mean[:, 0:1],
                                                   in1=w_bc[:], op0=OP.subtract, op1=OP.mult)
                    nc.vector.scalar_tensor_tensor(out=u[:], in0=u[:], scalar=rstd[:, 0:1],
                                                   in1=b_bc[:], op0=OP.mult, op1=OP.add)
                    ngt = pc.tile([128, fdim], dt.float32, tag=f"ng{fdim}")
                    nc.vector.tensor_scalar(out=ngt[:], in0=u[:], scalar1=0.0,
                                            scalar2=0.01, op0=OP.min, op1=OP.mult)
                    nc.vector.scalar_tensor_tensor(out=u[:], in0=u[:], scalar=0.0,
                                                   in1=ngt[:], op0=OP.max, op1=OP.add)
                    return u

                def transpose_to_b(xin, pdim, fdim):
                    # xin [pdim, fdim] f32 -> bf16 sbuf [fdim, pdim]
                    p = pc_ps.tile([128, 128], dt.float32, tag="tpp", space="PSUM")
                    nc.tensor.transpose(out=p[:fdim, 0:pdim], in_=xin,
                                        identity=ident[:pdim, :pdim])
                    s = pc.tile([fdim, pdim], dt.bfloat16, tag=f"tt{fdim}_{pdim}")
                    nc.scalar.copy(out=s[:], in_=p[:fdim, 0:pdim])
                    return s

                for b in range(RB):
                    x0 = xgs[b]
                    xt0c = transpose_to_b(x0[:, 0:128], 128, 128)
                    xt1c = transpose_to_b(x0[:, 128:256], 128, 128)
                    pA = pc_ps.tile([128, 128], dt.float32, tag="mm", space="PSUM")
                    nc.tensor.matmul(pA[:], xt0c[:], waT0[:], start=True, stop=False)
                    nc.tensor.matmul(pA[:], xt1c[:], waT1[:], start=False, stop=True)
                    x1 = layer_norm_lrelu(pA[:], 128, ba_bc, lnaw_bc, lnab_bc)

                    x1t = transpose_to_b(x1[:], 128, 128)
                    p1 = pc_ps.tile([128, 64], dt.float32, tag="mm", space="PSUM")
                    nc.tensor.matmul(p1[:], x1t[:], w1T[:], start=True, stop=True)
                    x2 = layer_norm_lrelu(p1[:], 64, b1_bc, ln1w_bc, ln1b_bc)

                    x2t = transpose_to_b(x2[:], 128, 64)
                    p2 = pc_ps.tile([128, 32], dt.float32, tag="mm", space="PSUM")
                    nc.tensor.matmul(p2[:], x2t[:], w2T[:], start=True, stop=True)
                    x3 = layer_norm_lrelu(p2[:], 32, b2_bc, ln2w_bc, ln2b_bc)

                    x3t = transpose_to_b(x3[:], 128, 32)
                    p3 = pc_ps.tile([128, 3], dt.float32, tag="mm", space="PSUM")
                    nc.tensor.matmul(p3[:], x3t[:], w3T[:], start=True, stop=True)
                    y3 = pc.tile([128, 3], dt.float32, tag="y3")
                    nc.vector.tensor_tensor(out=y3[:], in0=p3[:], in1=b3_bc[:], op=OP.add)
                    # x~ = bf16(y3); sq = sum x~^2 (f32); split sq = s~ + r~ in bf16
                    y3b = pc.tile([128, 3], dt.bfloat16, tag="y3b")
                    nc.vector.tensor_copy(out=y3b[:], in_=y3[:])
                    scr3 = pc.tile([128, 3], dt.float32, tag="scr3")
                    sq = pc.tile([128, 1], dt.float32, tag="sq")
                    nc.scalar.activation(out=scr3[:], in_=y3b[:], func=AF.Square,
                                         accum_out=sq[:])
                    sqb = pc.tile([128, 1], dt.bfloat16, tag="sqb")
                    nc.vector.tensor_copy(out=sqb[:], in_=sq[:])
                    sqbf = pc.tile([128, 1], dt.float32, tag="sqbf")
                    nc.vector.tensor_copy(out=sqbf[:], in_=sqb[:])
                    resid = pc.tile([128, 1], dt.float32, tag="resid")
                    nc.vector.tensor_tensor(out=resid[:], in0=sq[:], in1=sqbf[:],
                                            op=OP.subtract)
                    # y3e cols: [-2x~ (0:3) | s~ (3) | r~ (4) | 1 (5) | 1 (6) | 0 (7)]
                    y3e = pc.tile([128, 8], dt.float32, tag="y3e")
                    nc.scalar.activation(out=y3e[:, 0:3], in_=y3b[:], func=AF.Copy,
                                         scale=-2.0)
                    nc.scalar.copy(out=y3e[:, 3:4], in_=sqb[:])
                    nc.vector.tensor_copy(out=y3e[:, 4:5], in_=resid[:])
                    nc.vector.memset(y3e[:, 5:7], 1.0)
                    nc.vector.memset(y3e[:, 7:8], 0.0)
                    # cct cols: [x~ (0:3) | s~ (3) | r~ (4) | 0...]
                    cct = pc.tile([128, 8], dt.float32, tag="cct")
                    nc.scalar.copy(out=cct[:, 0:3], in_=y3b[:])
                    nc.scalar.copy(out=cct[:, 3:4], in_=sqb[:])
                    nc.vector.tensor_copy(out=cct[:, 4:5], in_=resid[:])
                    nc.vector.memset(cct[:, 5:8], 0.0)
                    h3p = pc_ps.tile([128, 128], dt.float32, tag="tpp", space="PSUM")
                    nc.tensor.transpose(out=h3p[:8, 0:128], in_=cct[:], identity=ident[:])
                    nc.scalar.copy(out=cc_sb[:, b * 128:(b + 1) * 128], in_=h3p[:5, 0:128])
                    h3q = pc_ps.tile([128, 128], dt.float32, tag="tpp", space="PSUM")
                    nc.tensor.transpose(out=h3q[:8, 0:128], in_=y3e[:], identity=ident[:])
                    lt = lhsT_pool.tile([7, 128], dt.bfloat16, tag=f"lt{b}")
                    nc.scalar.copy(out=lt[:], in_=h3q[:7, 0:128])
                    lhsTs.append(lt)

            if phases == "ABC":
                nc.sync.dma_start(out=outD[0:5, 0:RPAD], in_=cc_sb[:])
                raise _PhaseDone()

            # ---- phase D: allgather [x~ | s~ | r~] ----
            nc.sync.dma_start(out=cc_in[:], in_=cc_sb[:])
            nc.gpsimd.collective_compute(
                "AllGather", OP.bypass, replica_groups=[list(range(CORES))],
                ins=[cc_in[:].opt()], outs=[cc_out[:].opt()])
            # rhs rows: [x~ (0:3) | 1 (3) | 1 (4) | s~ (5) | r~ (6)]
            rhs_all = top.tile([7, NCOL], dt.bfloat16, tag="rhs_all")
            nc.vector.memset(rhs_all[:], 1.0)
            for s in range(CORES):
                c0 = s * SHARD
                nc.sync.dma_start(out=rhs_all[0:3, c0:c0 + SHARD],
                                  in_=cc_out[:][s, 0:3, 0:SHARD])
                nc.sync.dma_start(out=rhs_all[5:7, c0:c0 + SHARD],
                                  in_=cc_out[:][s, 3:5, 0:SHARD])

            if phases == "ABCD":
                nc.sync.dma_start(out=outD[0:7, 0:NCOL], in_=rhs_all[:])
                raise _PhaseDone()

            # ---- phase E: cdist row-block x col-chunk ----
            with tc.tile_pool(name="pe_d", bufs=6) as pe_d, \
                 tc.tile_pool(name="pe_d2", bufs=3) as pe_d2, \
                 tc.tile_pool(name="pe_ps", bufs=6, space="PSUM") as pe_ps:
                for rb in range(RB):
                    for mg in range(NCOL // (CCH * MRG)):
                        d2t = pe_d2.tile([128, CCH * MRG], dt.bfloat16, tag="d2t")
                        for k in range(MRG):
                            ch = mg * MRG + k
                            dp = pe_ps.tile([128, CCH], dt.float32, tag="dp", space="PSUM")
                            nc.tensor.matmul(dp[:], lhsTs[rb][:],
                                             rhs_all[:, ch * CCH:(ch + 1) * CCH],
                                             start=True, stop=True)
                            dtl = pe_d.tile([128, CCH], dt.bfloat16, tag="dtl")
                            nc.vector.tensor_scalar(out=dtl[:], in0=dp[:], scalar1=0.0,
                                                    scalar2=None, op0=OP.max)
                            nc.scalar.activation(out=d2t[:, k * CCH:(k + 1) * CCH],
                                                 in_=dtl[:], func=AF.Sqrt)
                        nc.sync.dma_start(
                            out=outD[rb * 128:(rb + 1) * 128,
                                     mg * CCH * MRG:(mg + 1) * CCH * MRG],
                            in_=d2t[:])

    except _PhaseDone:
        pass
    _tc_cm.__exit__(None, None, None)
    nc.compile()
    return nc


def _prep_host(x, edge_index):
    xp = np.zeros((NPAD, FIN), np.float32)
    xp[:N] = np.asarray(x, np.float32)
    xp = np.ascontiguousarray(xp.T).astype(BF16)  # [256, NPAD] bf16

    ei = np.asarray(edge_index)
    src = np.concatenate([ei[0], np.arange(N, dtype=np.int64)]).astype(np.int64)
    dst = np.concatenate([ei[1], np.arange(N, dtype=np.int64)]).astype(np.int64)

    core = dst // SHARD
    loc = dst - core * SHARD
    blk = loc // 128
    lane = loc - blk * 128

    per_core = []
    max_tiles = 1
    for c in range(CORES):
        blocks = []
        selc = core == c
        s_c, b_c, l_c = src[selc], blk[selc], lane[selc]
        for b in range(RB):
            m = b_c == b
            blocks.append((s_c[m], l_c[m]))
            max_tiles = max(max_tiles, (len(blocks[-1][0]) + 127) // 128)
        per_core.append(blocks)

    TB = max_tiles

    hidx = np.zeros((CORES, RB, 16, TB * 8), np.int16)
    dstl = np.full((CORES, RB, 128, TB), 255.0, np.float32)
    for c in range(CORES):
        for b in range(RB):
            s_b, l_b = per_core[c][b]
            n = len(s_b)
            js = np.arange(n)
            hidx[c, b, js % 16, js // 16] = s_b.astype(np.int16)
            dstl[c, b, js % 128, js // 128] = l_b.astype(np.float32)
    hidx = np.tile(hidx, (1, 1, 8, 1))
    dstrow = np.ascontiguousarray(
        dstl.transpose(0, 1, 3, 2)).reshape(CORES, RB, 1, TB * 128).astype(BF16)

    # adrw: own dst rows per block (c*1280 + b*128 + jr), wrapped in 16 partitions
    adrw = np.zeros((CORES, RB, 16, 8), np.int16)
    jr = np.arange(128)
    for c in range(CORES):
        for b in range(RB):
            adrw[c, b, jr % 16, jr // 16] = (c * SHARD + b * 128 + jr).astype(np.int16)
    adrw = np.tile(adrw, (1, 1, 8, 1))
    return xp, hidx, adrw, dstl, dstrow, TB


def build_in_maps(inputs):
    xp, hidx, adrw, dstl, dstrow, TB = _prep_host(inputs["x"], inputs["edge_index"])
    f32 = lambda a: np.ascontiguousarray(np.asarray(a, np.float32))
    row = lambda a: f32(a).reshape(1, -1)
    shared = {
        "xt": xp, "wg": f32(inputs["W_gat"]),
        "att_src": f32(inputs["att_src"]), "att_dst": f32(inputs["att_dst"]),
        "bgat": row(inputs["b_gat"]), "wa": f32(inputs["Wa"]), "ba": row(inputs["ba"]),
        "lnaw": row(inputs["lna_w"]), "lnab": row(inputs["lna_b"]),
        "w1": f32(inputs["W1"]), "b1": row(inputs["b1"]),
        "ln1w": row(inputs["ln1_w"]), "ln1b": row(inputs["ln1_b"]),
        "w2": f32(inputs["W2"]), "b2": row(inputs["b2"]),
        "ln2w": row(inputs["ln2_w"]), "ln2b": row(inputs["ln2_b"]),
        "w3": f32(inputs["W3"]), "b3": row(inputs["b3"]),
    }
    in_maps = [
        {**shared, "hidx": np.ascontiguousarray(hidx[c]),
         "adrw": np.ascontiguousarray(adrw[c]),
         "dstloc": np.ascontiguousarray(dstl[c]),
         "dstrow": np.ascontiguousarray(dstrow[c])}
        for c in range(CORES)
    ]
    return in_maps, TB


def kernel(**inputs):
    in_maps, TB = build_in_maps(inputs)

    import os
    phases = os.environ.get("K_PHASES", "ABCDE")
    key = (TB, phases)
    if key not in _BUILD_CACHE:
        _BUILD_CACHE[key] = _build(TB, phases)
    nc = _BUILD_CACHE[key]
    res = run_bass_kernel_spmd(nc, in_maps, core_ids=list(range(CORES)))
    global _LAST_RESULTS
    _LAST_RESULTS = res.results
    out = np.empty((N, N), np.float32)
    for c in range(CORES):
        r0 = c * SHARD
        r1 = min(N, r0 + SHARD)
        out[r0:r1, :] = res.results[c]["outD"][0:r1 - r0, :N].astype(np.float32)
    return out


# revision 21
# speedup vs baseline: 1.4129x; 1.4129x over previous
"""GAT (2-head) + 3x dense/LayerNorm + pairwise-distance kernel for 8 TRN2 NeuronCores.

Strategy: dst-sharded edge processing (one-hot matmul aggregation with the
softmax weights applied on the matmul RHS), bf16 gather table and matmuls,
replicated small dense weights, row-block-sharded NxN cdist in bf16 with an
exact split-fp32 sq trick, bf16 output converted to f32 on host.
"""
import sys

import numpy as np

# Environment bootstrap (harness may run from a bare directory).
for _p in ("/root/.axon_site", "/root/.axon_site/_ro/trn_rl_repo",
           "/root/.axon_site/_ro/pypackages", "/opt/trn_rl_repo"):
    if _p not in sys.path:
        sys.path.append(_p)

import ml_dtypes
import concourse.bass as bass
import concourse.bacc as bacc
import concourse.mybir as mybir
import concourse.tile as tile
from concourse.masks import make_identity
from concourse.bass_utils import run_bass_kernel_spmd

dt = mybir.dt
OP = mybir.AluOpType
AF = mybir.ActivationFunctionType
BF16 = ml_dtypes.bfloat16

N = 10000
NPAD = 10240          # 80 * 128
NB = 80               # node blocks (phase A)
FIN = 256
F = 128               # per-head GAT dim
H = 2
HROW = 384            # htable row bf16 elems (768B, multiple of 256B)
CORES = 8
SHARD = 1280          # dst rows per core (core 7: 1040 useful)
RB = 10               # dst blocks per core
RPAD = 1280
CCH = 512             # cdist column chunk
MRG = 4               # cdist chunks merged per output DMA
NCOL = 10240          # padded output columns
GCMAX = 6             # max tiles per gather call (768 idxs = 48 desc/engine; >768 wedges)
EPS = 1e-5

_BUILD_CACHE = {}
_LAST_RESULTS = None


def _chunks(TB):
    out = []
    t = TB
    while t > 0:
        g = min(GCMAX, t)
        out.append(g)
        t -= g
    return out


def _build(TB, phases="ABCDE"):
    CHUNKS = _chunks(TB)

    nc = bacc.Bacc("TRN2", target_bir_lowering=False, debug=False, num_devices=CORES)

    def din(name, shape, d=dt.float32):
        return nc.dram_tensor(name, shape, d, kind="ExternalInput").ap()

    xt = din("xt", [128, 2, NPAD], dt.bfloat16)
    wg = din("wg", [FIN, FIN])
    att_src = din("att_src", [H, F])
    att_dst = din("att_dst", [H, F])
    bgat = din("bgat", [1, FIN])
    wa = din("wa", [128, 256])
    ba = din("ba", [1, 128])
    lnaw = din("lnaw", [1, 128])
    lnab = din("lnab", [1, 128])
    w1 = din("w1", [64, 128])
    b1 = din("b1", [1, 64])
    ln1w = din("ln1w", [1, 64])
    ln1b = din("ln1b", [1, 64])
    w2 = din("w2", [32, 64])
    b2 = din("b2", [1, 32])
    ln2w = din("ln2w", [1, 32])
    ln2b = din("ln2b", [1, 32])
    w3 = din("w3", [3, 32])
    b3 = din("b3", [1, 3])
    hidx = din("hidx", [RB, 128, TB * 8], dt.int16)
    adrw = din("adrw", [RB, 128, 8], dt.int16)
    dstloc = din("dstloc", [RB, 128, TB])
    dstrow = din("dstrow", [RB, 1, TB * 128], dt.bfloat16)
    outD = nc.dram_tensor("outD", [RPAD, NCOL], dt.bfloat16, kind="ExternalOutput").ap()

    class _PhaseDone(Exception):
        pass

    import contextlib
    try:
        _tc_cm = tile.TileContext(nc)
        tc = _tc_cm.__enter__()
        est = contextlib.ExitStack()
        with est:
            top = est.enter_context(tc.tile_pool(name="top", bufs=1))
            dram = est.enter_context(tc.tile_pool(name="dram", bufs=1, space="DRAM"))

            htable = dram.tile([NPAD, HROW], dt.bfloat16, tag="htable")
            cc_in = dram.tile([5, RPAD], dt.bfloat16, tag="cc_in")
            cc_out = dram.tile([CORES, 5, RPAD], dt.bfloat16, tag="cc_out")

            ident = top.tile([128, 128], dt.float32, tag="ident")
            make_identity(nc, ident[:])
            iota_i = top.tile([128, 128], dt.int32, tag="iota_i")
            nc.gpsimd.iota(iota_i[:], pattern=[[1, 128]], base=0, channel_multiplier=0)
            iota_f = top.tile([128, 128], dt.float32, tag="iota_f")
            nc.vector.tensor_copy(out=iota_f[:], in_=iota_i[:])
            ones_row = top.tile([1, 128], dt.float32, tag="ones_row")
            nc.vector.memset(ones_row[:], 1.0)
            ones_row_b = top.tile([1, 128], dt.bfloat16, tag="ones_row_b")
            nc.vector.memset(ones_row_b[:], 1.0)
            iotac_i = top.tile([128, 1], dt.int32, tag="iotac_i")
            nc.gpsimd.iota(iotac_i[:], pattern=[[1, 1]], base=0, channel_multiplier=1)
            iota_c = top.tile([128, 1], dt.float32, tag="iota_c")
            nc.vector.tensor_copy(out=iota_c[:], in_=iotac_i[:])
            eps_col = top.tile([128, 1], dt.float32, tag="eps_col")
            nc.vector.memset(eps_col[:], EPS)

            # ---- weight prep ----
            with tc.tile_pool(name="wprep", bufs=1) as wp, \
                 tc.tile_pool(name="wpsum", bufs=2, space="PSUM") as wps:
                wg0 = wp.tile([128, 256], dt.float32, tag="wg0")
                wg1 = wp.tile([128, 256], dt.float32, tag="wg1")
                nc.sync.dma_start(out=wg0[:], in_=wg[0:128, :])
                nc.sync.dma_start(out=wg1[:], in_=wg[128:256, :])
                # rhs chunks [128, 260] bf16: cols 0:256 = WgT[k,:], 256:260 = [as0,as1,ad0,ad1]
                rhsA0 = top.tile([128, 260], dt.bfloat16, tag="rhsA0")
                rhsA1 = top.tile([128, 260], dt.bfloat16, tag="rhsA1")
                for (ko, rhs_t) in ((0, rhsA0), (1, rhsA1)):
                    for oo, wgt in ((0, wg0), (1, wg1)):
                        p = wps.tile([128, 128], dt.float32, tag="tp", space="PSUM")
                        nc.tensor.transpose(out=p[:], in_=wgt[:, ko * 128:(ko + 1) * 128],
                                            identity=ident[:])
                        nc.scalar.copy(out=rhs_t[:, oo * 128:(oo + 1) * 128], in_=p[:])
                # attention fold: wtil[c, k] = sum_j attdiag[j, c] * Wg[j, k]
                asb = wp.tile([2, 128], dt.float32, tag="asb")
                adb = wp.tile([2, 128], dt.float32, tag="adb")
                nc.sync.dma_start(out=asb[:], in_=att_src[:])
                nc.sync.dma_start(out=adb[:], in_=att_dst[:])
                asT = wps.tile([128, 2], dt.float32, tag="attp", space="PSUM")
                nc.tensor.transpose(out=asT[:], in_=asb[:], identity=ident[:2, :2])
                adT = wps.tile([128, 2], dt.float32, tag="attp", space="PSUM")
                nc.tensor.transpose(out=adT[:], in_=adb[:], identity=ident[:2, :2])
                attd0 = wp.tile([128, 4], dt.float32, tag="attd0")
                attd1 = wp.tile([128, 4], dt.float32, tag="attd1")
                nc.vector.memset(attd0[:], 0.0)
                nc.vector.memset(attd1[:], 0.0)
                nc.scalar.copy(out=attd0[:, 0:1], in_=asT[:, 0:1])
                nc.scalar.copy(out=attd0[:, 2:3], in_=adT[:, 0:1])
                nc.scalar.copy(out=attd1[:, 1:2], in_=asT[:, 1:2])
                nc.scalar.copy(out=attd1[:, 3:4], in_=adT[:, 1:2])
                wtilp = wps.tile([4, 256], dt.float32, tag="wtilp", space="PSUM")
                nc.tensor.matmul(wtilp[:], attd0[:], wg0[:], start=True, stop=False)
                nc.tensor.matmul(wtilp[:], attd1[:], wg1[:], start=False, stop=True)
                wtil = wp.tile([4, 256], dt.float32, tag="wtil")
                nc.scalar.copy(out=wtil[:], in_=wtilp[:])
                for ko, rhs_t in ((0, rhsA0), (1, rhsA1)):
                    p = wps.tile([128, 4], dt.float32, tag="attp", space="PSUM")
                    nc.tensor.transpose(out=p[:], in_=wtil[:, ko * 128:(ko + 1) * 128],
                                        identity=ident[:4, :4])
                    nc.scalar.copy(out=rhs_t[:, 256:260], in_=p[:])

                # dense weight transposes (bf16)
                wa_sb = wp.tile([128, 256], dt.float32, tag="wa_sb")
                nc.sync.dma_start(out=wa_sb[:], in_=wa[:])
                waT0 = top.tile([128, 128], dt.bfloat16, tag="waT0")
                waT1 = top.tile([128, 128], dt.bfloat16, tag="waT1")
                for oo, dst_t in ((0, waT0), (1, waT1)):
                    p = wps.tile([128, 128], dt.float32, tag="tp", space="PSUM")
                    nc.tensor.transpose(out=p[:], in_=wa_sb[:, oo * 128:(oo + 1) * 128],
                                        identity=ident[:])
                    nc.scalar.copy(out=dst_t[:], in_=p[:])
                w1_sb = wp.tile([64, 128], dt.float32, tag="w1_sb")
                nc.sync.dma_start(out=w1_sb[:], in_=w1[:])
                w1T = top.tile([128, 64], dt.bfloat16, tag="w1T")
                p = wps.tile([128, 128], dt.float32, tag="tp", space="PSUM")
                nc.tensor.transpose(out=p[:, 0:64], in_=w1_sb[:], identity=ident[:64, :64])
                nc.scalar.copy(out=w1T[:], in_=p[:, 0:64])
                w2_sb = wp.tile([32, 64], dt.float32, tag="w2_sb")
                nc.sync.dma_start(out=w2_sb[:], in_=w2[:])
                w2T = top.tile([64, 32], dt.bfloat16, tag="w2T")
                p = wps.tile([128, 128], dt.float32, tag="tp", space="PSUM")
                nc.tensor.transpose(out=p[:64, 0:32], in_=w2_sb[:], identity=ident[:32, :32])
                nc.scalar.copy(out=w2T[:], in_=p[:64, 0:32])
                w3_sb = wp.tile([3, 32], dt.float32, tag="w3_sb")
                nc.sync.dma_start(out=w3_sb[:], in_=w3[:])
                w3T = top.tile([32, 3], dt.bfloat16, tag="w3T")
                p = wps.tile([128, 128], dt.float32, tag="tp", space="PSUM")
                nc.tensor.transpose(out=p[:32, 0:3], in_=w3_sb[:], identity=ident[:3, :3])
                nc.scalar.copy(out=w3T[:], in_=p[:32, 0:3])

                # broadcast bias / LN tiles (f32)
                def bcast(vec_ap, n, tag):
                    t = top.tile([128, n], dt.float32, tag=tag)
                    v = wp.tile([1, n], dt.float32, tag="bvec_" + tag)
                    nc.sync.dma_start(out=v[:], in_=vec_ap)
                    p2 = wps.tile([128, 256], dt.float32, tag="bp", space="PSUM")
                    nc.tensor.matmul(p2[:, 0:n], ones_row[:], v[:], start=True, stop=True)
                    nc.scalar.copy(out=t[:], in_=p2[:, 0:n])
                    return t

                bgat_bc = bcast(bgat[:], 256, "bgat_bc")
                ba_bc = bcast(ba[:], 128, "ba_bc")
                lnaw_bc = bcast(lnaw[:], 128, "lnaw_bc")
                lnab_bc = bcast(lnab[:], 128, "lnab_bc")
                b1_bc = bcast(b1[:], 64, "b1_bc")
                ln1w_bc = bcast(ln1w[:], 64, "ln1w_bc")
                ln1b_bc = bcast(ln1b[:], 64, "ln1b_bc")
                b2_bc = bcast(b2[:], 32, "b2_bc")
                ln2w_bc = bcast(ln2w[:], 32, "ln2w_bc")
                ln2b_bc = bcast(ln2b[:], 32, "ln2b_bc")
                b3_bc = bcast(b3[:], 3, "b3_bc")

            # ---- phase A: htable = [h0 | 1 | h1 | 1 | a_s | a_d | junk] per node ----
            # 3 rotating ht staging tiles with the constant cols pre-set
            hts = []
            for k in range(3):
                t = top.tile([128, HROW], dt.bfloat16, tag=f"htst{k}")
                nc.vector.memset(t[:, 128:129], 1.0)
                nc.vector.memset(t[:, 257:258], 1.0)
                nc.vector.memset(t[:, 262:HROW], 0.0)
                hts.append(t)
            with tc.tile_pool(name="pa", bufs=3) as pa, \
                 tc.tile_pool(name="pa_h", bufs=2, space="PSUM") as pa_h:
                for i in range(NB):
                    r0 = i * 128
                    xtt = pa.tile([128, 2, 128], dt.bfloat16, tag="xtt")
                    nc.sync.dma_start(out=xtt[:], in_=xt[:, :, r0:r0 + 128])
                    hp = pa_h.tile([128, 260], dt.float32, tag="hp", space="PSUM")
                    nc.tensor.matmul(hp[:], xtt[:, 0, :], rhsA0[:], start=True, stop=False)
                    nc.tensor.matmul(hp[:], xtt[:, 1, :], rhsA1[:], start=False, stop=True)
                    ht = hts[i % 3]
                    nc.scalar.copy(out=ht[:, 0:128], in_=hp[:, 0:128])
                    nc.vector.tensor_copy(out=ht[:, 129:257], in_=hp[:, 128:256])
                    nc.scalar.copy(out=ht[:, 258:262], in_=hp[:, 256:260])
                    nc.scalar.dma_start(out=htable[r0:r0 + 128, :], in_=ht[:])

            if phases == "A":
                dbg = top.tile([128, HROW], dt.bfloat16, tag="dbgA")
                for i in range(RB):
                    nc.sync.dma_start(out=dbg[:], in_=htable[i * 128:(i + 1) * 128, :])
                    nc.sync.dma_start(out=outD[i * 128:(i + 1) * 128, 0:HROW], in_=dbg[:])
                raise _PhaseDone()

            # ---- phase B: GAT aggregation per dst block ----
            SKIP_AD = "1" in phases
            xg_pool = est.enter_context(tc.tile_pool(name="xg", bufs=1))
            xgs = []

            with tc.tile_pool(name="pb_idx", bufs=2) as pb_idx, \
                 tc.tile_pool(name="pb_g", bufs=3) as pb_g, \
                 tc.tile_pool(name="pb_ex", bufs=3) as pb_ex, \
                 tc.tile_pool(name="pb_oh", bufs=2) as pb_oh, \
                 tc.tile_pool(name="pb_oh2", bufs=4) as pb_oh2, \
                 tc.tile_pool(name="pb_gs", bufs=8) as pb_gs, \
                 tc.tile_pool(name="pb_ep", bufs=2) as pb_ep, \
                 tc.tile_pool(name="pb_ps", bufs=2, space="PSUM") as pb_ps, \
                 tc.tile_pool(name="pb_bc", bufs=2, space="PSUM") as pb_bc, \
                 tc.tile_pool(name="pb_adp", bufs=2, space="PSUM") as pb_adp:
                for b in range(RB):
                    hix = pb_idx.tile([128, TB * 8], dt.int16, tag="hix")
                    nc.sync.dma_start(out=hix[:], in_=hidx[b])
                    dloc = pb_idx.tile([128, TB], dt.float32, tag="dloc")
                    nc.sync.dma_start(out=dloc[:], in_=dstloc[b])
                    drow = pb_idx.tile([1, TB * 128], dt.bfloat16, tag="drow")
                    nc.sync.dma_start(out=drow[:], in_=dstrow[b])
                    adblk = pb_idx.tile([128, 1, HROW], dt.bfloat16, tag="adblk")
                    if SKIP_AD:
                        nc.vector.memset(adblk[:], 0.0)
                    else:
                        arw = pb_idx.tile([128, 8], dt.int16, tag="arw")
                        nc.sync.dma_start(out=arw[:], in_=adrw[b])
                        nc.gpsimd.dma_gather(
                            out_ap=adblk[:], in_ap=htable[:],
                            idxs_ap=arw[:], num_idxs=128, num_idxs_reg=128,
                            elem_size=HROW)

                    ps = pb_ps.tile([128, 258], dt.float32, tag="ps", space="PSUM")

                    t0 = 0
                    for gc in CHUNKS:
                        g = pb_g.tile([128, GCMAX, HROW], dt.bfloat16, tag="g")
                        nc.gpsimd.dma_gather(
                            out_ap=g[:, 0:gc, :], in_ap=htable[:],
                            idxs_ap=hix[:, t0 * 8:(t0 + gc) * 8],
                            num_idxs=128 * gc, num_idxs_reg=128 * gc,
                            elem_size=HROW)
                        exr = pb_ex.tile([128, GCMAX, 2], dt.float32, tag="exr")
                        if SKIP_AD:
                            nc.vector.tensor_copy(out=exr[:, 0:gc, :],
                                                  in_=g[:, 0:gc, 258:260])
                        else:
                            # ohT[dstlane, edgecol] one-hots for the a_d fetch
                            ohT = pb_oh.tile([128, GCMAX * 128], dt.bfloat16, tag="ohT")
                            for s0 in range(0, gc * 128, 512):
                                w = min(512, gc * 128 - s0)
                                bcp = pb_bc.tile([128, 512], dt.float32, tag="bcp",
                                                 space="PSUM")
                                nc.tensor.matmul(bcp[:, 0:w], ones_row_b[:],
                                                 drow[0:1, t0 * 128 + s0:t0 * 128 + s0 + w],
                                                 start=True, stop=True)
                                nc.vector.tensor_scalar(
                                    out=ohT[:, s0:s0 + w], in0=bcp[:, 0:w],
                                    scalar1=iota_c[:, 0:1], scalar2=None, op0=OP.is_equal)
                            # a_d per edge via permutation matmuls into one psum tile
                            adp = pb_adp.tile([128, GCMAX, 2], dt.float32, tag="adp",
                                              space="PSUM")
                            for tl in range(gc):
                                nc.tensor.matmul(adp[:, tl, :],
                                                 ohT[:, tl * 128:(tl + 1) * 128],
                                                 adblk[:, 0, 260:262],
                                                 start=True, stop=True)
                            asf = pb_ex.tile([128, GCMAX, 2], dt.float32, tag="asf")
                            nc.vector.tensor_copy(out=asf[:, 0:gc, :],
                                                  in_=g[:, 0:gc, 258:260])
                            nc.vector.tensor_tensor(out=exr[:, 0:gc, :],
                                                    in0=asf[:, 0:gc, :],
                                                    in1=adp[:, 0:gc, :], op=OP.add)
                        exn = pb_ex.tile([128, GCMAX, 2], dt.float32, tag="exn")
                        nc.scalar.activation(out=exn[:, 0:gc, :], in_=exr[:, 0:gc, :],
                                             func=AF.Exp, scale=0.2)
                        exp_ = pb_ex.tile([128, GCMAX, 2], dt.float32, tag="exp_")
                        nc.scalar.activation(out=exp_[:, 0:gc, :], in_=exr[:, 0:gc, :],
                                             func=AF.Exp)
                        exv = pb_ex.tile([128, GCMAX, 2], dt.float32, tag="exv")
                        nc.vector.tensor_tensor(out=exv[:, 0:gc, :], in0=exp_[:, 0:gc, :],
                                                in1=exn[:, 0:gc, :], op=OP.max)

                        for tl in range(gc):
                            t = t0 + tl
                            gs = pb_gs.tile([128, 258], dt.bfloat16, tag="gs")
                            nc.scalar.activation(out=gs[:, 0:129], in_=g[:, tl, 0:129],
                                                 func=AF.Copy, scale=exv[:, tl, 0:1])
                            nc.vector.tensor_scalar(
                                out=gs[:, 129:258], in0=g[:, tl, 129:258],
                                scalar1=exv[:, tl, 1:2], scalar2=None, op0=OP.mult)
                            oh = pb_oh2.tile([128, 128], dt.bfloat16, tag="oh")
                            nc.vector.tensor_scalar(
                                out=oh[:], in0=iota_f[:], scalar1=dloc[:, t:t + 1],
                                scalar2=None, op0=OP.is_equal)
                            nc.tensor.matmul(ps[:], oh[:], gs[:],
                                             start=(t == 0), stop=(t == TB - 1))
                        t0 += gc

                    # epilogue: normalize, +b_gat, lrelu(0.01)
                    rec0 = pb_ep.tile([128, 1], dt.float32, tag="rec0")
                    nc.vector.reciprocal(rec0[:], ps[:, 128:129])
                    rec1 = pb_ep.tile([128, 1], dt.float32, tag="rec1")
                    nc.vector.reciprocal(rec1[:], ps[:, 257:258])
                    xg = xg_pool.tile([128, 256], dt.float32, tag=f"xg{b}")
                    nc.scalar.activation(out=xg[:, 0:128], in_=ps[:, 0:128],
                                         func=AF.Copy, scale=rec0[:])
                    nc.scalar.activation(out=xg[:, 128:256], in_=ps[:, 129:257],
                                         func=AF.Copy, scale=rec1[:])
                    nc.vector.tensor_tensor(out=xg[:], in0=xg[:], in1=bgat_bc[:], op=OP.add)
                    ng = pb_ep.tile([128, 256], dt.float32, tag="ng")
                    nc.vector.tensor_scalar(out=ng[:], in0=xg[:], scalar1=0.0,
                                            scalar2=0.01, op0=OP.min, op1=OP.mult)
                    nc.vector.scalar_tensor_tensor(out=xg[:], in0=xg[:], scalar=0.0,
                                                   in1=ng[:], op0=OP.max, op1=OP.add)
                    xgs.append(xg)

            if phases.startswith("AB") and "C" not in phases:
                for b in range(RB):
                    xgb_d = top.tile([128, 256], dt.bfloat16, tag=f"dbgB{b}")
                    nc.vector.tensor_copy(out=xgb_d[:], in_=xgs[b][:])
                    nc.sync.dma_start(out=outD[b * 128:(b + 1) * 128, 0:256], in_=xgb_d[:])
                raise _PhaseDone()

            # ---- phase C: dense + LN (layer-major across the 10 blocks) ----
            cc_sb = top.tile([5, RPAD], dt.bfloat16, tag="cc_sb")
            lhsT_pool = est.enter_context(tc.tile_pool(name="lhsT", bufs=1))
            lhsTs = []
            with tc.tile_pool(name="pc", bufs=1) as pc, \
                 tc.tile_pool(name="pc_tp", bufs=3, space="PSUM") as pc_tp, \
                 tc.tile_pool(name="pc_ps", bufs=4, space="PSUM") as pc_ps:

                def transpose_all(xins, pdim, fdim, tag):
                    outs = []
                    for b, xin in enumerate(xins):
                        p = pc_tp.tile([128, 128], dt.float32, tag="tpp", space="PSUM")
                        nc.tensor.transpose(out=p[:fdim, 0:pdim], in_=xin,
                                            identity=ident[:pdim, :pdim])
                        s = pc.tile([fdim, pdim], dt.bfloat16, tag=f"{tag}{b}")
                        nc.scalar.copy(out=s[:], in_=p[:fdim, 0:pdim])
                        outs.append(s)
                    return outs

                def dense_ln_all(xts, fin_tiles, fdim, wts, bias_bc, w_bc, b_bc, lname):
                    # xts: list of lhsT tiles [128(k), 128(node)]; wts: list of rhs
                    ys, means, m2s, vars_, rstds, sqss = [], [], [], [], [], []
                    for b in range(RB):
                        pm = pc_ps.tile([128, fdim], dt.float32, tag="mm", space="PSUM")
                        first = True
                        for (xt_t, w_t) in zip(xts[b], wts):
                            nc.tensor.matmul(pm[:], xt_t[:], w_t[:], start=first,
                                             stop=(w_t is wts[-1]))
                            first = False
                        y = pc.tile([128, fdim], dt.float32, tag=f"y{lname}{b}")
                        nc.vector.tensor_tensor(out=y[:], in0=pm[:], in1=bias_bc[:],
                                                op=OP.add)
                        ys.append(y)
                    for b in range(RB):
                        scr = pc.tile([128, fdim], dt.float32, tag=f"scr{lname}{b}")
                        msum = pc.tile([128, 1], dt.float32, tag=f"ms{lname}{b}")
                        nc.scalar.activation(out=scr[:], in_=ys[b][:], func=AF.Copy,
                                             accum_out=msum[:])
                        means.append(msum)
                        sqs = pc.tile([128, 1], dt.float32, tag=f"sq{lname}{b}")
                        nc.scalar.activation(out=scr[:], in_=ys[b][:], func=AF.Square,
                                             accum_out=sqs[:])
                        sqss.append(sqs)
                    for b in range(RB):
                        mean = pc.tile([128, 1], dt.float32, tag=f"mn{lname}{b}")
                        nc.vector.tensor_scalar(out=mean[:], in0=means[b][:],
                                                scalar1=1.0 / fdim, scalar2=None,
                                                op0=OP.mult)
                        means[b] = mean
                        m2 = pc.tile([128, 1], dt.float32, tag=f"m2{lname}{b}")
                        nc.vector.tensor_scalar(out=m2[:], in0=mean[:],
                                                scalar1=mean[:, 0:1], scalar2=None,
                                                op0=OP.mult)
                        m2s.append(m2)
                    for b in range(RB):
                        var = pc.tile([128, 1], dt.float32, tag=f"vr{lname}{b}")
                        nc.vector.scalar_tensor_tensor(out=var[:], in0=sqss[b][:],
                                                       scalar=1.0 / fdim, in1=m2s[b][:],
                                                       op0=OP.mult, op1=OP.subtract)
                        sd = pc.tile([128, 1], dt.float32, tag=f"sd{lname}{b}")
                        nc.scalar.activation(out=sd[:], in_=var[:], func=AF.Sqrt,
                                             bias=eps_col[:, 0:1])
                        rstd = pc.tile([128, 1], dt.float32, tag=f"rs{lname}{b}")
                        nc.vector.reciprocal(rstd[:], sd[:])
                        rstds.append(rstd)
                    us = []
                    for b in range(RB):
                        u = pc.tile([128, fdim], dt.float32, tag=f"u{lname}{b}")
                        nc.vector.scalar_tensor_tensor(out=u[:], in0=ys[b][:],
                                                       scalar=means[b][:, 0:1],
                                                       in1=w_bc[:], op0=OP.subtract,
                                                       op1=OP.mult)
                        nc.vector.scalar_tensor_tensor(out=u[:], in0=u[:],
                                                       scalar=rstds[b][:, 0:1],
                                                       in1=b_bc[:], op0=OP.mult,
                                                       op1=OP.add)
                        ng_ = pc.tile([128, fdim], dt.float32, tag=f"ng{lname}{b}")
                        nc.vector.tensor_scalar(out=ng_[:], in0=u[:], scalar1=0.0,
                                                scalar2=0.01, op0=OP.min, op1=OP.mult)
                        nc.vector.scalar_tensor_tensor(out=u[:], in0=u[:], scalar=0.0,
                                                       in1=ng_[:], op0=OP.max, op1=OP.add)
                        us.append(u)
                    return us

                xtAs = []
                for b in range(RB):
                    pair = transpose_all([xgs[b][:, 0:128], xgs[b][:, 128:256]],
                                         128, 128, f"xtA{b}_")
                    xtAs.append(pair)
                x1s = dense_ln_all(xtAs, None, 128, [waT0, waT1], ba_bc, lnaw_bc,
                                   lnab_bc, "A")
                x1ts = transpose_all([x[:] for x in x1s], 128, 128, "x1t")
                x2s = dense_ln_all([[t] for t in x1ts], None, 64, [w1T], b1_bc,
                                   ln1w_bc, ln1b_bc, "B")
                x2ts = transpose_all([x[:] for x in x2s], 128, 64, "x2t")
                x3s = dense_ln_all([[t] for t in x2ts], None, 32, [w2T], b2_bc,
                                   ln2w_bc, ln2b_bc, "C")
                x3ts = transpose_all([x[:] for x in x3s], 128, 32, "x3t")

                for b in range(RB):
                    p3 = pc_ps.tile([128, 3], dt.float32, tag="mm", space="PSUM")
                    nc.tensor.matmul(p3[:], x3ts[b][:], w3T[:], start=True, stop=True)
                    y3 = pc.tile([128, 3], dt.float32, tag=f"y3_{b}")
                    nc.vector.tensor_tensor(out=y3[:], in0=p3[:], in1=b3_bc[:], op=OP.add)
                    # x~ = bf16(y3); sq = sum x~^2 (f32); split sq = s~ + r~ in bf16
                    y3b = pc.tile([128, 3], dt.bfloat16, tag=f"y3b{b}")
                    nc.vector.tensor_copy(out=y3b[:], in_=y3[:])
                    scr3 = pc.tile([128, 3], dt.float32, tag=f"sc3{b}")
                    sq = pc.tile([128, 1], dt.float32, tag=f"sqq{b}")
                    nc.scalar.activation(out=scr3[:], in_=y3b[:], func=AF.Square,
                                         accum_out=sq[:])
                    sqb = pc.tile([128, 1], dt.bfloat16, tag=f"sqb{b}")
                    nc.vector.tensor_copy(out=sqb[:], in_=sq[:])
                    sqbf = pc.tile([128, 1], dt.float32, tag=f"sbf{b}")
                    nc.vector.tensor_copy(out=sqbf[:], in_=sqb[:])
                    resid = pc.tile([128, 1], dt.float32, tag=f"rsd{b}")
                    nc.vector.tensor_tensor(out=resid[:], in0=sq[:], in1=sqbf[:],
                                            op=OP.subtract)
                    # combined cols: [-2x~ (0:3) | s~ (3) | r~ (4) | 1 (5) | 1 (6)
                    #                 | x~ (32:35) | s~ (35) | r~ (36)]
                    y3e = pc.tile([128, 40], dt.float32, tag=f"y3e{b}")
                    nc.scalar.activation(out=y3e[:, 0:3], in_=y3b[:], func=AF.Copy,
                                         scale=-2.0)
                    nc.scalar.copy(out=y3e[:, 3:4], in_=sqb[:])
                    nc.vector.tensor_copy(out=y3e[:, 4:5], in_=resid[:])
                    nc.vector.memset(y3e[:, 5:7], 1.0)
                    nc.scalar.copy(out=y3e[:, 32:35], in_=y3b[:])
                    nc.scalar.copy(out=y3e[:, 35:36], in_=sqb[:])
                    nc.vector.tensor_copy(out=y3e[:, 36:37], in_=resid[:])
                    h3q = pc_tp.tile([128, 128], dt.float32, tag="tpp", space="PSUM")
                    nc.tensor.transpose(out=h3q[:40, 0:128], in_=y3e[:], identity=ident[:])
                    lt = lhsT_pool.tile([7, 128], dt.bfloat16, tag=f"lt{b}")
                    nc.scalar.copy(out=lt[:], in_=h3q[:7, 0:128])
                    lhsTs.append(lt)
                    nc.scalar.copy(out=cc_sb[:, b * 128:(b + 1) * 128],
                                   in_=h3q[32:37, 0:128])

            if phases == "ABC":
                nc.sync.dma_start(out=outD[0:5, 0:RPAD], in_=cc_sb[:])
                raise _PhaseDone()

            # ---- phase D: allgather [x~ | s~ | r~] ----
            nc.sync.dma_start(out=cc_in[:], in_=cc_sb[:])
            nc.gpsimd.collective_compute(
                "AllGather", OP.bypass, replica_groups=[list(range(CORES))],
                ins=[cc_in[:].opt()], outs=[cc_out[:].opt()])
            # rhs rows: [x~ (0:3) | 1 (3) | 1 (4) | s~ (5) | r~ (6)]
            rhs_all = top.tile([7, NCOL], dt.bfloat16, tag="rhs_all")
            nc.vector.memset(rhs_all[:], 1.0)
            for s in range(CORES):
                c0 = s * SHARD
                nc.sync.dma_start(out=rhs_all[0:3, c0:c0 + SHARD],
                                  in_=cc_out[:][s, 0:3, 0:SHARD])
                nc.sync.dma_start(out=rhs_all[5:7, c0:c0 + SHARD],
                                  in_=cc_out[:][s, 3:5, 0:SHARD])

            if phases == "ABCD":
                nc.sync.dma_start(out=outD[0:7, 0:NCOL], in_=rhs_all[:])
                raise _PhaseDone()

            # ---- phase E: cdist row-block x col-chunk ----
            with tc.tile_pool(name="pe_d", bufs=4) as pe_d, \
                 tc.tile_pool(name="pe_d2", bufs=3) as pe_d2, \
                 tc.tile_pool(name="pe_ps", bufs=3, space="PSUM") as pe_ps:
                for rb in range(RB):
                    for mg in range(NCOL // (CCH * MRG)):
                        d2t = pe_d2.tile([128, CCH * MRG], dt.bfloat16, tag="d2t")
                        dtl = pe_d.tile([128, CCH * MRG], dt.bfloat16, tag="dtl")
                        for h2 in range(MRG // 2):
                            dp = pe_ps.tile([128, 2 * CCH], dt.float32, tag="dp",
                                            space="PSUM")
                            for k2 in range(2):
                                ch = mg * MRG + h2 * 2 + k2
                                nc.tensor.matmul(dp[:, k2 * CCH:(k2 + 1) * CCH],
                                                 lhsTs[rb][:],
                                                 rhs_all[:, ch * CCH:(ch + 1) * CCH],
                                                 start=True, stop=True)
                            if h2 % 2 == 0:
                                nc.vector.tensor_scalar(
                                    out=dtl[:, h2 * 2 * CCH:(h2 + 1) * 2 * CCH],
                                    in0=dp[:], scalar1=0.0, scalar2=None, op0=OP.max)
                            else:
                                nc.scalar.activation(
                                    out=dtl[:, h2 * 2 * CCH:(h2 + 1) * 2 * CCH],
                                    in_=dp[:], func=AF.Relu)
                        nc.scalar.activation(out=d2t[:], in_=dtl[:], func=AF.Sqrt)
                        nc.sync.dma_start(
                            out=outD[rb * 128:(rb + 1) * 128,
                                     mg * CCH * MRG:(mg + 1) * CCH * MRG],
                            in_=d2t[:])

    except _PhaseDone:
        pass
    _tc_cm.__exit__(None, None, None)
    nc.compile()
    return nc


def _prep_host(x, edge_index):
    xp = np.zeros((NPAD, FIN), np.float32)
    xp[:N] = np.asarray(x, np.float32)
    # [128, 2, NPAD]: (k, half) -> feature half*128+k
    xp = np.ascontiguousarray(
        xp.T.reshape(2, 128, NPAD).transpose(1, 0, 2)).astype(BF16)

    ei = np.asarray(edge_index)
    src = np.concatenate([ei[0], np.arange(N, dtype=np.int64)]).astype(np.int64)
    dst = np.concatenate([ei[1], np.arange(N, dtype=np.int64)]).astype(np.int64)

    core = dst // SHARD
    loc = dst - core * SHARD
    blk = loc // 128
    lane = loc - blk * 128

    per_core = []
    max_tiles = 1
    for c in range(CORES):
        blocks = []
        selc = core == c
        s_c, b_c, l_c = src[selc], blk[selc], lane[selc]
        for b in range(RB):
            m = b_c == b
            blocks.append((s_c[m], l_c[m]))
            max_tiles = max(max_tiles, (len(blocks[-1][0]) + 127) // 128)
        per_core.append(blocks)

    TB = max_tiles

    hidx = np.zeros((CORES, RB, 16, TB * 8), np.int16)
    dstl = np.full((CORES, RB, 128, TB), 255.0, np.float32)
    for c in range(CORES):
        for b in range(RB):
            s_b, l_b = per_core[c][b]
            n = len(s_b)
            js = np.arange(n)
            hidx[c, b, js % 16, js // 16] = s_b.astype(np.int16)
            dstl[c, b, js % 128, js // 128] = l_b.astype(np.float32)
    hidx = np.tile(hidx, (1, 1, 8, 1))
    dstrow = np.ascontiguousarray(
        dstl.transpose(0, 1, 3, 2)).reshape(CORES, RB, 1, TB * 128).astype(BF16)

    # adrw: own dst rows per block (c*1280 + b*128 + jr), wrapped in 16 partitions
    adrw = np.zeros((CORES, RB, 16, 8), np.int16)
    jr = np.arange(128)
    for c in range(CORES):
        for b in range(RB):
            adrw[c, b, jr % 16, jr // 16] = (c * SHARD + b * 128 + jr).astype(np.int16)
    adrw = np.tile(adrw, (1, 1, 8, 1))
    return xp, hidx, adrw, dstl, dstrow, TB


def build_in_maps(inputs):
    xp, hidx, adrw, dstl, dstrow, TB = _prep_host(inputs["x"], inputs["edge_index"])
    f32 = lambda a: np.ascontiguousarray(np.asarray(a, np.float32))
    row = lambda a: f32(a).reshape(1, -1)
    shared = {
        "xt": xp, "wg": f32(inputs["W_gat"]),
        "att_src": f32(inputs["att_src"]), "att_dst": f32(inputs["att_dst"]),
        "bgat": row(inputs["b_gat"]), "wa": f32(inputs["Wa"]), "ba": row(inputs["ba"]),
        "lnaw": row(inputs["lna_w"]), "lnab": row(inputs["lna_b"]),
        "w1": f32(inputs["W1"]), "b1": row(inputs["b1"]),
        "ln1w": row(inputs["ln1_w"]), "ln1b": row(inputs["ln1_b"]),
        "w2": f32(inputs["W2"]), "b2": row(inputs["b2"]),
        "ln2w": row(inputs["ln2_w"]), "ln2b": row(inputs["ln2_b"]),
        "w3": f32(inputs["W3"]), "b3": row(inputs["b3"]),
    }
    in_maps = [
        {**shared, "hidx": np.ascontiguousarray(hidx[c]),
         "adrw": np.ascontiguousarray(adrw[c]),
         "dstloc": np.ascontiguousarray(dstl[c]),
         "dstrow": np.ascontiguousarray(dstrow[c])}
        for c in range(CORES)
    ]
    return in_maps, TB


def kernel(**inputs):
    in_maps, TB = build_in_maps(inputs)

    import os
    phases = os.environ.get("K_PHASES", "ABCDE")
    key = (TB, phases)
    if key not in _BUILD_CACHE:
        _BUILD_CACHE[key] = _build(TB, phases)
    nc = _BUILD_CACHE[key]
    res = run_bass_kernel_spmd(nc, in_maps, core_ids=list(range(CORES)))
    global _LAST_RESULTS
    _LAST_RESULTS = res.results
    out = np.empty((N, N), np.float32)
    for c in range(CORES):
        r0 = c * SHARD
        r1 = min(N, r0 + SHARD)
        out[r0:r1, :] = res.results[c]["outD"][0:r1 - r0, :N].astype(np.float32)
    return out
